# revision 31
# baseline (speedup 1.0000x reference)
"""DeepSeekMoE Trainium2 kernel: 8-way expert-parallel, host-routed dispatch.

v3: all matmul operands bf16 (fp32 PSUM), contiguous host-prepped DMA,
host-computed gate weights, and stationary-weight reuse everywhere: each
LDWEIGHTS feeds 2-4 matmuls (measured ~245 -> ~216-231 ns per 512-wide MM on
this part), via k-outer loops accumulating several PSUM chunks at once.

Layout notes (per core e):
  - routed expert e computes its assigned tokens (gathered, zero-padded to
    `cap`) as xet [P, DK, cap] bf16; contraction dim D on SBUF partitions.
  - shared SwiGLU is tensor-parallel over the inter dim (352-wide slice,
    padded to 384); token halves are processed as quarter-pairs so layer-1
    stationaries feed two moving quarters.
  - wcol [P, capm] f32 carries each token's routing weight (host softmax).
Host combines: y = sum_e ypart_e; y[idx_e] += yrouted_e[:cnt_e].
"""
import numpy as np

import concourse.bass as bass
import concourse.mybir as mybir
import concourse.tile as tile
from concourse import bacc
from concourse.bass_utils import run_bass_kernel_spmd

D = 2048
F = 1408
E = 8
TOPK = 2
FSL = 352             # per-core shared slice (F * N_SHARED / 8)
FSP = 384             # padded to 3*128
NT = 2048             # tokens (2*1024)
P = 128
DK = D // P           # 16
FK = F // P           # 11
SK = FSP // P         # 3
NQ = 4                # token quarters for the shared expert
QW = NT // NQ         # 512
DCH = 4               # D output chunks of 512
F32 = mybir.dt.float32
BF16 = mybir.dt.bfloat16
NPBF16 = mybir.dt.np(mybir.dt.bfloat16)
SILU = mybir.ActivationFunctionType.Silu
IDENT = mybir.ActivationFunctionType.Identity

_nc_cache: dict[tuple, object] = {}


def _l1_chunks(cap):
    assert cap <= 1024
    if cap <= 512:
        return [(0, cap)]
    return [(0, 512), (512, cap - 512)]


def _build(cap: int, repeat: int = 1):
    """SPMD program for per-expert token capacity `cap` (multiple of 128).

    repeat>1 re-runs the whole body (same inputs -> same outputs); used only
    to amortize the fixed per-dispatch cost when timing.
    """
    from contextlib import ExitStack
    capm = cap // P
    cap_chunks = _l1_chunks(cap)
    NCH = len(cap_chunks)

    nc = bacc.Bacc("TRN2", target_bir_lowering=False)
    xet = nc.declare_dram_parameter("xet", [P, DK, cap], BF16, isOutput=False)
    rw1 = nc.declare_dram_parameter("rw1", [P, FK, DK, P], BF16, isOutput=False)
    rw3 = nc.declare_dram_parameter("rw3", [P, FK, DK, P], BF16, isOutput=False)
    rw2 = nc.declare_dram_parameter("rw2", [P, FK, DK, P], BF16,
                                    isOutput=False)
    swa = nc.declare_dram_parameter("swa", [P, DK, FSP], BF16, isOutput=False)
    swb = nc.declare_dram_parameter("swb", [P, DK, FSP], BF16, isOutput=False)
    swc = nc.declare_dram_parameter("swc", [P, SK, D], BF16, isOutput=False)
    xt = nc.declare_dram_parameter("xt", [P, NQ, DK, QW], BF16, isOutput=False)
    # routed layer-2 output transposed: [D, tokens] tiled by D
    yrouted = nc.declare_dram_parameter("yrouted", [DK, P, cap], BF16,
                                        isOutput=True)
    ypart = nc.declare_dram_parameter("ypart", [NT // P, P, D], BF16,
                                      isOutput=True)

    with tile.TileContext(nc) as tc, ExitStack() as es:
        res_pool = es.enter_context(tc.tile_pool(name="res", bufs=1))
        SWA = res_pool.tile([P, DK, FSP], BF16)
        SWB = res_pool.tile([P, DK, FSP], BF16)
        SWC = res_pool.tile([P, SK, D], BF16)
        xtq_pool = es.enter_context(tc.tile_pool(name="sh_xt", bufs=3))

        for _ in range(repeat):
            xtq = []

            # ---- routed phase ----
            with tc.tile_pool(name="gt", bufs=1) as gt_pool, \
                 tc.tile_pool(name="w2res", bufs=1) as w2_pool, \
                 tc.tile_pool(name="stage_rt", bufs=3) as stage, \
                 tc.tile_pool(name="ostage_rt", bufs=3) as ostage:
                GT = gt_pool.tile([P, FK, cap], BF16)
                W2 = w2_pool.tile([P, FK, DK, P], BF16)

                # layer 1: gT = silu(x@w1) * (x@w3); k-outer, one stationary
                # feeds every cap chunk
                with tc.tile_pool(name="rt1x", bufs=1) as xet_pool, \
                     tc.tile_pool(name="rt1w", bufs=2) as wpool1, \
                     tc.tile_pool(name="psum_l1", bufs=2,
                                  space="PSUM") as psum1:
                    # first weight tiles on the scalar queue: the first
                    # matmul only waits for w1c + XET[k=0]
                    w1c0 = wpool1.tile([P, DK, P], BF16, tag="w1c")
                    nc.scalar.dma_start(w1c0[:], rw1[:, 0])
                    w3c0 = wpool1.tile([P, DK, P], BF16, tag="w3c")
                    nc.scalar.dma_start(w3c0[:], rw3[:, 0])
                    XET = xet_pool.tile([P, DK, cap], BF16)
                    for k in range(DK):
                        nc.sync.dma_start(XET[:, k], xet[:, k])
                    for m in range(FK):
                        if m == 0:
                            w1c, w3c = w1c0, w3c0
                        else:
                            w1c = wpool1.tile([P, DK, P], BF16, tag="w1c")
                            nc.sync.dma_start(w1c[:], rw1[:, m])
                            w3c = wpool1.tile([P, DK, P], BF16, tag="w3c")
                            nc.sync.dma_start(w3c[:], rw3[:, m])
                        if m >= 4 and m < 8:
                            j0 = 3 * (m - 4)
                            j1 = min(FK, j0 + 3)
                            nc.sync.dma_start(W2[:, j0:j1], rw2[:, j0:j1])
                        if m == 7:
                            nc.sync.dma_start(SWA[:], swa[:])
                            nc.sync.dma_start(SWB[:], swb[:])
                        if m == 8:
                            nc.sync.dma_start(SWC[:], swc[:])
                        if m >= 9:
                            # prefetch shared-phase token quarters 0..1
                            XTQn = xtq_pool.tile([P, DK, QW], BF16,
                                                 tag="xtq",
                                                 name=f"xtq{m - 9}")
                            nc.sync.dma_start(XTQn[:], xt[:, m - 9])
                            xtq.append(XTQn)

                        psa = [psum1.tile([P, 512], F32, tag=f"a{j}",
                                          name=f"psa{j}")
                               for j in range(NCH)]
                        for k in range(DK):
                            for j, (n0, nw) in enumerate(cap_chunks):
                                nc.tensor.matmul(
                                    psa[j][:, :nw], w1c[:, k],
                                    XET[:, k, n0:n0 + nw],
                                    start=(k == 0), stop=(k == DK - 1))
                        sa = stage.tile([P, cap], F32, tag="rt1_silu")
                        for j, (n0, nw) in enumerate(cap_chunks):
                            nc.scalar.activation(sa[:, n0:n0 + nw],
                                                 psa[j][:, :nw], SILU)
                        psb = [psum1.tile([P, 512], F32, tag=f"b{j}",
                                          name=f"psb{j}")
                               for j in range(NCH)]
                        for k in range(DK):
                            for j, (n0, nw) in enumerate(cap_chunks):
                                nc.tensor.matmul(
                                    psb[j][:, :nw], w3c[:, k],
                                    XET[:, k, n0:n0 + nw],
                                    start=(k == 0), stop=(k == DK - 1))
                        for j, (n0, nw) in enumerate(cap_chunks):
                            nc.vector.tensor_mul(GT[:, m, n0:n0 + nw],
                                                 sa[:, n0:n0 + nw],
                                                 psb[j][:, :nw])

                # layer 2 (transposed): yT_e[d, t] = rw2T_d . gT_t; cycles
                # scale with cap, routing weight applied on host. One
                # stationary w2 tile feeds every cap chunk.
                with tc.tile_pool(name="psum_l2", bufs=2,
                                  space="PSUM") as psum2:
                    for dt in range(DK):
                        ot = ostage.tile([P, cap], BF16, tag="rt2_out")
                        ps = [psum2.tile([P, 512], F32, tag=f"r{j}",
                                         name=f"psr{j}")
                              for j in range(NCH)]
                        for k in range(FK):
                            for j, (n0, nw) in enumerate(cap_chunks):
                                nc.tensor.matmul(
                                    ps[j][:, :nw], W2[:, k, dt],
                                    GT[:, k, n0:n0 + nw],
                                    start=(k == 0), stop=(k == FK - 1))
                        for j, (n0, nw) in enumerate(cap_chunks):
                            if j % 2 == 0:
                                nc.vector.tensor_copy(ot[:, n0:n0 + nw],
                                                      ps[j][:, :nw])
                            else:
                                nc.scalar.activation(ot[:, n0:n0 + nw],
                                                     ps[j][:, :nw], IDENT)
                        if dt % 2 == 0:
                            nc.scalar.dma_start(yrouted[dt], ot[:])
                        else:
                            nc.sync.dma_start(yrouted[dt], ot[:])

            # ---- shared expert (TP slice over inter dim), quarter-pairs ----
            with tc.tile_pool(name="gst", bufs=1) as gst_pool, \
                 tc.tile_pool(name="stage_sh", bufs=3) as stage_sh, \
                 tc.tile_pool(name="ostage_sh", bufs=3) as ostage_sh, \
                 tc.tile_pool(name="psum_sh1", bufs=1, space="PSUM") as psum_sh1, \
                 tc.tile_pool(name="psum_sh2", bufs=1, space="PSUM") as psum_sh2:
                GST = gst_pool.tile([P, SK, NT], BF16)
                # quarter 2 prefetch behind 0/1 (loaded in routed phase)
                XTQn = xtq_pool.tile([P, DK, QW], BF16, tag="xtq",
                                     name="xtq2")
                nc.sync.dma_start(XTQn[:], xt[:, 2])
                xtq.append(XTQn)

                for pair in range(NQ // 2):
                    qa, qb = 2 * pair, 2 * pair + 1
                    XA, XB = xtq[qa], xtq[qb]
                    for m in range(SK):
                        psa = [psum_sh1.tile([P, 512], F32, tag=f"sa{j}",
                                             name=f"pssa{j}")
                               for j in range(2)]
                        for k in range(DK):
                            nc.tensor.matmul(psa[0][:], SWA[:, k, bass.ts(m, P)],
                                             XA[:, k], start=(k == 0),
                                             stop=(k == DK - 1))
                            nc.tensor.matmul(psa[1][:], SWA[:, k, bass.ts(m, P)],
                                             XB[:, k], start=(k == 0),
                                             stop=(k == DK - 1))
                        sa = stage_sh.tile([P, 2 * QW], F32, tag="sh1_silu")
                        nc.scalar.activation(sa[:, :QW], psa[0][:], SILU)
                        nc.scalar.activation(sa[:, QW:], psa[1][:], SILU)
                        psb = [psum_sh1.tile([P, 512], F32, tag=f"sb{j}",
                                             name=f"pssb{j}")
                               for j in range(2)]
                        for k in range(DK):
                            nc.tensor.matmul(psb[0][:], SWB[:, k, bass.ts(m, P)],
                                             XA[:, k], start=(k == 0),
                                             stop=(k == DK - 1))
                            nc.tensor.matmul(psb[1][:], SWB[:, k, bass.ts(m, P)],
                                             XB[:, k], start=(k == 0),
                                             stop=(k == DK - 1))
                        nc.vector.tensor_mul(GST[:, m, qa * QW:(qa + 1) * QW],
                                             sa[:, :QW], psb[0][:])
                        nc.vector.tensor_mul(GST[:, m, qb * QW:(qb + 1) * QW],
                                             sa[:, QW:], psb[1][:])

                    # layer 2 for this pair's tokens; prefetch quarter 3
                    if pair == 0:
                        XTQn = xtq_pool.tile([P, DK, QW], BF16, tag="xtq",
                                             name="xtq3")
                        nc.sync.dma_start(XTQn[:], xt[:, 3])
                        xtq.append(XTQn)
                    for mt in range(qa * QW // P, (qb + 1) * QW // P):
                        ot = ostage_sh.tile([P, D], BF16, tag="sh2_out")
                        ps = [psum_sh2.tile([P, 512], F32, tag=f"s{ci}",
                                            name=f"pss{ci}")
                              for ci in range(DCH)]
                        for k in range(SK):
                            for ci in range(DCH):
                                nc.tensor.matmul(
                                    ps[ci][:], GST[:, k, bass.ts(mt, P)],
                                    SWC[:, k, bass.ts(ci, 512)],
                                    start=(k == 0), stop=(k == SK - 1))
                        for ci in range(DCH):
                            if ci % 2 == 0:
                                nc.vector.tensor_copy(ot[:, bass.ts(ci, 512)],
                                                      ps[ci][:])
                            else:
                                nc.scalar.activation(ot[:, bass.ts(ci, 512)],
                                                     ps[ci][:], IDENT)
                        if mt % 2 == 0:
                            nc.scalar.dma_start(ypart[mt], ot[:])
                        else:
                            nc.sync.dma_start(ypart[mt], ot[:])

    nc.compile()
    return nc


def _route(xf: np.ndarray, gate_w: np.ndarray):
    logits = xf @ gate_w
    m = logits.max(-1, keepdims=True)
    ex = np.exp(logits - m)
    scores = ex / ex.sum(-1, keepdims=True)
    top2 = np.argsort(-scores, axis=-1)[:, :TOPK]
    return scores, top2


def _route_idx(xf, gate_w):
    scores, top2 = _route(xf, gate_w)
    idx = [np.where((top2 == e).any(axis=1))[0] for e in range(E)]
    return idx, scores


def build_in_maps(inputs, cap):
    """Per-core device input maps for capacity `cap` (shared with timing)."""
    x = np.asarray(inputs["x"], dtype=np.float32)
    xf = np.ascontiguousarray(x.reshape(-1, D))
    gate_w = np.asarray(inputs["gate_w"], dtype=np.float32)
    idx, scores = _route_idx(xf, gate_w)

    xfb = xf.astype(NPBF16)
    # [P, NQ, DK, QW]: element [p, q, k, u] = xf[q*QW+u, k*128+p]
    xt_b = np.ascontiguousarray(
        xfb.reshape(NQ, QW, DK, P).transpose(3, 0, 2, 1))

    sw1 = np.asarray(inputs["sw1"], dtype=np.float32)
    sw2 = np.asarray(inputs["sw2"], dtype=np.float32)
    sw3 = np.asarray(inputs["sw3"], dtype=np.float32)
    rw1 = np.asarray(inputs["rw1"], dtype=np.float32)
    rw2 = np.asarray(inputs["rw2"], dtype=np.float32)
    rw3 = np.asarray(inputs["rw3"], dtype=np.float32)

    in_maps = []
    for e in range(E):
        ie = idx[e]
        cnt = len(ie)
        # xet [P, DK, cap]: [p, k, c] = xf[ie[c], k*128+p]
        xet = np.zeros((P, DK, cap), dtype=NPBF16)
        xet[:, :, :cnt] = xfb[ie].reshape(cnt, DK, P).transpose(2, 1, 0)
        # rw1/rw3 [P, FK, DK, P]: [p, m, k, c] = rw[k*128+p, m*128+c]
        rw1_b = np.ascontiguousarray(
            rw1[e].astype(NPBF16).reshape(DK, P, FK, P).transpose(1, 2, 0, 3))
        rw3_b = np.ascontiguousarray(
            rw3[e].astype(NPBF16).reshape(DK, P, FK, P).transpose(1, 2, 0, 3))
        # rw2 [P, FK, DK, P]: [p, k, n, c] = rw2[k*128+p, n*128+c]
        rw2_b = np.ascontiguousarray(
            rw2[e].astype(NPBF16).reshape(FK, P, DK, P).transpose(1, 0, 2, 3))
        # swa/swb [P, DK, FSP]: [p, k, c] = sw[k*128+p, e*FSL+c] (pad c>=FSL)
        za = np.zeros((D, FSP), dtype=NPBF16)
        za[:, :FSL] = sw1[:, e * FSL:(e + 1) * FSL]
        swa_b = np.ascontiguousarray(
            za.reshape(DK, P, FSP).transpose(1, 0, 2))
        zb = np.zeros((D, FSP), dtype=NPBF16)
        zb[:, :FSL] = sw3[:, e * FSL:(e + 1) * FSL]
        swb_b = np.ascontiguousarray(
            zb.reshape(DK, P, FSP).transpose(1, 0, 2))
        # swc [P, SK, D]: [p, k, c] = sw2[e*FSL + k*128+p, c] (pad)
        zc = np.zeros((FSP, D), dtype=NPBF16)
        zc[:FSL] = sw2[e * FSL:(e + 1) * FSL]
        swc_b = np.ascontiguousarray(
            zc.reshape(SK, P, D).transpose(1, 0, 2))
        in_maps.append({
            "xet": xet,
            "rw1": rw1_b, "rw3": rw3_b, "rw2": rw2_b,
            "swa": swa_b, "swb": swb_b, "swc": swc_b, "xt": xt_b,
        })
    return in_maps, (idx, scores)


LAST_RESULTS = None


def kernel(x, gate_w, sw1, sw2, sw3, rw1, rw2, rw3, _trace=False):
    x = np.asarray(x, dtype=np.float32)
    B, T, _ = x.shape
    xf = np.ascontiguousarray(x.reshape(-1, D))
    gate_w = np.asarray(gate_w, dtype=np.float32)
    idx, _ = _route_idx(xf, gate_w)
    maxcnt = max(len(i) for i in idx)
    cap = max(512, -(-maxcnt // 64) * 64)
    if cap not in _nc_cache:
        _nc_cache[cap] = _build(cap)
    nc = _nc_cache[cap]

    in_maps, (idx, scores) = build_in_maps(
        {"x": x, "gate_w": gate_w, "sw1": sw1, "sw2": sw2, "sw3": sw3,
         "rw1": rw1, "rw2": rw2, "rw3": rw3}, cap)

    res = run_bass_kernel_spmd(nc, in_maps, list(range(E)), trace=_trace)
    global LAST_RESULTS
    LAST_RESULTS = res

    y = res.results[0]["ypart"].astype(np.float32).reshape(NT, D)
    for e in range(1, E):
        y += res.results[e]["ypart"].astype(np.float32).reshape(NT, D)
    for e in range(E):
        ie = idx[e]
        cnt = len(ie)
        # yrouted is [D, cap] (transposed); apply routing weights here
        yr = res.results[e]["yrouted"].astype(np.float32).reshape(D, cap)
        y[ie] += yr[:, :cnt].T * scores[ie, e][:, None]
    return y.reshape(B, T, D)
